# revision 9
# baseline (speedup 1.0000x reference)
"""Trainium2 Bass kernel for a 2-layer GRU stack (nn_GRU_43112881717881).

Model (see reference): logits = sm_fc(GRU2(GRU1(in_fc(x)))), PyTorch gate
order (r, z, n), dp_keep_prob = 1 so dropouts are identity.

Strategy (8 NeuronCores, data-parallel over batch):
  - batch 128 is sharded 8 ways (16 per core); each core runs its whole
    pipeline independently — no cross-core communication.
  - The input projection in_fc folds into layer 0's gate GEMM on the host:
    xp0 = x @ (W_ih[0] @ in_fc_w)^T, contraction K=64.
  - Per layer: xp = gate-input projections for all timesteps (one big GEMM),
    then the sequential recurrence: per step hp = h @ W_hh^T as 48 matmuls
    (8 K-tiles x 6 N-chunks of 512) with the transposed hidden state as the
    PE-stationary operand, float32r so the PE streams at 1 col/cycle.
    Gate math on DVE/ACT/GpSimd overlaps the PE stream; h is re-transposed
    each step with 8 PE transposes, which also produces the transposed
    layer output y^T consumed by the next layer's xp GEMM (and sm_fc).
  - All matmul accumulation in fp32 PSUM; everything stored fp32.

Self-contained: hardcodes shapes, includes the walrus sync-wait legalization.
"""

import numpy as np

import concourse.bass as bass
import concourse.mybir as mybir
from concourse.tile import TileContext
from concourse.bass_utils import run_bass_kernel_spmd
from concourse.masks import make_identity

# ----------------------------------------------------------------------------
# walrus sync-wait legalization: this walrus build accepts at most ONE sync
# wait per instruction; Tile attaches several. Hoist extras onto fresh
# same-engine NoOps placed immediately before the owner.
# ----------------------------------------------------------------------------
_nop_counter = [0]


def _dedupe_waits(waits):
    best = {}
    order = []
    for w in waits:
        key = (w.sync_type, w.id, w.wait_mode, w.wait_reg)
        if key in best:
            prev = best[key]
            if (
                w.wait_value is not None
                and prev.wait_value is not None
                and w.wait_value > prev.wait_value
            ):
                best[key] = w
        else:
            best[key] = w
            order.append(key)
    return [best[k] for k in order]


def legalize_sync_waits(nc):
    cap = 1
    for fn in nc.m.functions:
        for bb in fn.blocks:
            insts = list(bb.instructions)
            out = []
            changed = False
            for ins in insts:
                si = ins.sync_info
                if si is not None and si.on_wait:
                    waits = _dedupe_waits(list(si.on_wait))
                    if len(waits) > cap:
                        for w in waits[cap:]:
                            _nop_counter[0] += 1
                            out.append(
                                mybir.InstNoOp(
                                    name=f"I-waitnop-{_nop_counter[0]}",
                                    engine=ins.engine,
                                    ins=[],
                                    outs=[],
                                    sync_info=mybir.SyncInfo(
                                        on_wait=[w], on_update=[]
                                    ),
                                )
                            )
                        si.on_wait = waits[:cap]
                        changed = True
                    elif len(waits) != len(si.on_wait):
                        si.on_wait = waits
                out.append(ins)
            if changed:
                bb.instructions = out


def _patch_tile_scheduling():
    if getattr(TileContext, "_gru_patched", False):
        return
    orig = TileContext.schedule_and_allocate

    def wrapped(self):
        orig(self)
        import os
        if os.environ.get("GRU_NO_LEGALIZE", "0") != "1":
            legalize_sync_waits(self.nc)

    TileContext.schedule_and_allocate = wrapped
    TileContext._gru_patched = True


# ----------------------------------------------------------------------------
# shapes
# ----------------------------------------------------------------------------
S, B, I, H, L = 512, 128, 64, 1024, 2
NCORES = 8
BLOC = B // NCORES          # 16 batch rows per core
G = 3 * H                   # 3072 gate width
NK = H // 128               # 8 K-tiles
NNC = G // 512              # 6 N-chunks of 512
F32 = mybir.dt.float32
F32R = mybir.dt.float32r

# stream order: r chunks, n chunks, z chunks — so the n/r gate math can run
# under the z matmuls and the tail only holds the z-dependent update.
_CHUNK_ORDER = [0, 1, 4, 5, 2, 3]  # col0 = idx*512


def build_program(nsteps):
    rows = nsteps * BLOC
    nrt = rows // 128  # row tiles for the xp / logits GEMMs

    _patch_tile_scheduling()
    nc = bass.Bass("TRN2", target_bir_lowering=False, debug=False)

    xT = nc.declare_dram_parameter("xT", [I, rows], F32R, isOutput=False)
    w0effT = nc.declare_dram_parameter("w0effT", [I, G], F32R, isOutput=False)
    w1T = nc.declare_dram_parameter("w1T", [H, G], F32R, isOutput=False)
    whhT = nc.declare_dram_parameter("whhT", [L, H, G], F32R, isOutput=False)
    bias128 = nc.declare_dram_parameter("bias128", [L, 128, G], F32, isOutput=False)
    bhh_rep = nc.declare_dram_parameter("bhh_rep", [L, BLOC, H], F32, isOutput=False)
    h0 = nc.declare_dram_parameter("h0", [L, BLOC, H], F32, isOutput=False)
    h0T = nc.declare_dram_parameter("h0T", [L, H, BLOC], F32R, isOutput=False)
    smwT = nc.declare_dram_parameter("smwT", [H, I], F32R, isOutput=False)

    logits_o = nc.declare_dram_parameter("logits", [rows, I], F32, isOutput=True)
    hidden_o = nc.declare_dram_parameter("hidden", [L, BLOC, H], F32, isOutput=True)

    xp0_d = nc.dram_tensor("xp0_d", [rows, G], F32)
    xp1_d = nc.dram_tensor("xp1_d", [rows, G], F32)
    y0T_d = nc.dram_tensor("y0T_d", [NK, 128, rows], F32R)
    y1T_d = nc.dram_tensor("y1T_d", [NK, 128, rows], F32R)

    with TileContext(nc) as tc:
        # ------------------------------------------------------------------
        # phase A: xp0 = x @ w0eff^T + bias0   (K=64)
        # ------------------------------------------------------------------
        with (
            tc.tile_pool(name="a_const", bufs=1) as cpool,
            tc.tile_pool(name="a_stat", bufs=3) as spool,
            tc.tile_pool(name="a_ps", bufs=2, space="PSUM") as pspool,
            tc.tile_pool(name="a_out", bufs=3) as opool,
        ):
            w0_sb = cpool.tile([I, G], F32R)
            nc.sync.dma_start(out=w0_sb[:], in_=w0effT[:])
            bias_sb = cpool.tile([128, G], F32)
            nc.sync.dma_start(out=bias_sb[:], in_=bias128[0])
            for rt in range(nrt):
                st = spool.tile([I, 128], F32R)
                nc.sync.dma_start(out=st[:], in_=xT[:, rt * 128 : (rt + 1) * 128])
                for half in range(2):
                    ps = pspool.tile([128, G // 2], F32)
                    for j in range(NNC // 2):
                        col = half * (G // 2) + j * 512
                        nc.tensor.matmul(
                            ps[:, j * 512 : (j + 1) * 512],
                            st[:],
                            w0_sb[:, col : col + 512],
                            start=True,
                            stop=True,
                        )
                    ot = opool.tile([128, G // 2], F32, tag="a_out")
                    nc.vector.tensor_add(
                        ot[:], ps[:], bias_sb[:, half * (G // 2) : (half + 1) * (G // 2)]
                    )
                    nc.sync.dma_start(
                        out=xp0_d[
                            rt * 128 : (rt + 1) * 128,
                            half * (G // 2) : (half + 1) * (G // 2),
                        ],
                        in_=ot[:],
                    )

        # ------------------------------------------------------------------
        # recurrence phases (layer 0 then layer 1)
        # ------------------------------------------------------------------
        def recurrence(l, xp_d, yT_d):
            with (
                tc.tile_pool(name=f"r{l}_w", bufs=1) as wpool,
                tc.tile_pool(name=f"r{l}_const", bufs=1) as cpool,
                tc.tile_pool(name=f"r{l}_xp", bufs=2) as xpool,
                tc.tile_pool(name=f"r{l}_h", bufs=2) as hpool,
                tc.tile_pool(name=f"r{l}_yacc", bufs=2) as ypool,
                tc.tile_pool(name=f"r{l}_g", bufs=1) as gpool,
                tc.tile_pool(name=f"r{l}_ps", bufs=1, space="PSUM") as pspool,
                tc.tile_pool(name=f"r{l}_ptr", bufs=2, space="PSUM") as trpool,
            ):
                w_sb = wpool.tile([128, NK * G], F32R)
                for c in range(NK):
                    nc.sync.dma_start(
                        out=w_sb[:, c * G : (c + 1) * G],
                        in_=whhT[l, c * 128 : (c + 1) * 128, :],
                    )
                bhh_sb = cpool.tile([BLOC, H], F32)
                nc.sync.dma_start(out=bhh_sb[:], in_=bhh_rep[l])
                ident = cpool.tile([BLOC, BLOC], F32)
                make_identity(nc, ident[:])

                h_cur = hpool.tile([BLOC, H], F32, tag="h")
                nc.sync.dma_start(out=h_cur[:], in_=h0[l])
                hT0 = cpool.tile([128, 128], F32R)
                for c in range(NK):
                    nc.sync.dma_start(
                        out=hT0[:, c * BLOC : (c + 1) * BLOC],
                        in_=h0T[l, c * 128 : (c + 1) * 128, :],
                    )
                hT_cur = hT0[:, :]
                y_acc = None
                # h'(t)^T lands in y_acc col block (t % 8); a y_acc buffer is
                # flushed to DRAM as the y^T output every 8 steps.

                for t in range(nsteps):
                    xp_t = xpool.tile([BLOC, G], F32, tag="xp")
                    nc.sync.dma_start(
                        out=xp_t[:], in_=xp_d[t * BLOC : (t + 1) * BLOC, :]
                    )
                    ps = pspool.tile([BLOC, G], F32, tag="ps")
                    for idx in _CHUNK_ORDER:
                        col = idx * 512
                        for k in range(NK):
                            nc.tensor.matmul(
                                ps[:, col : col + 512],
                                hT_cur[:, k * BLOC : (k + 1) * BLOC],
                                w_sb[:, k * G + col : k * G + col + 512],
                                start=(k == 0),
                                stop=(k == NK - 1),
                            )
                    # gates: r = sig(xr+hr), z = sig(xz+hz), n = tanh(xn + r*(hn+bhh_n))
                    a_r = gpool.tile([BLOC, H], F32, tag="a_r")
                    nc.vector.tensor_add(a_r[:], ps[:, 0:H], xp_t[:, 0:H])
                    r = gpool.tile([BLOC, H], F32, tag="r")
                    nc.scalar.activation(
                        r[:], a_r[:], mybir.ActivationFunctionType.Sigmoid
                    )
                    hnb = gpool.tile([BLOC, H], F32, tag="hnb")
                    nc.vector.tensor_add(hnb[:], ps[:, 2 * H : 3 * H], bhh_sb[:])
                    t1 = gpool.tile([BLOC, H], F32, tag="t1")
                    nc.vector.tensor_mul(t1[:], r[:], hnb[:])
                    t2 = gpool.tile([BLOC, H], F32, tag="t2")
                    nc.vector.tensor_add(t2[:], t1[:], xp_t[:, 2 * H : 3 * H])
                    n = gpool.tile([BLOC, H], F32, tag="n")
                    nc.scalar.activation(
                        n[:], t2[:], mybir.ActivationFunctionType.Tanh
                    )
                    d = gpool.tile([BLOC, H], F32, tag="d")
                    nc.gpsimd.tensor_sub(d[:], h_cur[:], n[:])
                    a_z = gpool.tile([BLOC, H], F32, tag="a_z")
                    nc.vector.tensor_add(a_z[:], ps[:, H : 2 * H], xp_t[:, H : 2 * H])
                    z = gpool.tile([BLOC, H], F32, tag="z")
                    nc.scalar.activation(
                        z[:], a_z[:], mybir.ActivationFunctionType.Sigmoid
                    )
                    zd = gpool.tile([BLOC, H], F32, tag="zd")
                    nc.vector.tensor_mul(zd[:], z[:], d[:])
                    h_new = hpool.tile([BLOC, H], F32, tag="h")
                    nc.vector.tensor_add(h_new[:], n[:], zd[:])

                    # transpose h_new -> next stationary + y^T output block
                    blk = t % 8
                    if blk == 0:
                        y_acc = ypool.tile([128, 8 * 128], F32R, tag="yacc")
                    ptr = trpool.tile([128, 128], F32, tag="ptr")
                    for c in range(NK):
                        nc.tensor.transpose(
                            ptr[:, c * BLOC : (c + 1) * BLOC],
                            h_new[:, c * 128 : (c + 1) * 128],
                            ident[:],
                        )
                    hT_new = y_acc[:, blk * 128 : (blk + 1) * 128]
                    nc.scalar.copy(hT_new, ptr[:])
                    # flush y^T to DRAM once per 8 steps (steps t-blk .. t)
                    if blk == 7 or t == nsteps - 1:
                        t0 = t - blk
                        for c in range(NK):
                            nc.sync.dma_start(
                                out=yT_d[
                                    c, :, t0 * BLOC : (t + 1) * BLOC
                                ].rearrange("p (s b) -> p s b", b=BLOC),
                                in_=y_acc[:, 0 : (blk + 1) * 128]
                                .rearrange("p (s r) -> p s r", r=128)[
                                    :, :, c * BLOC : (c + 1) * BLOC
                                ],
                            )
                    h_cur, hT_cur = h_new, hT_new

                nc.sync.dma_start(out=hidden_o[l], in_=h_cur[:])

        recurrence(0, xp0_d, y0T_d)

        # ------------------------------------------------------------------
        # phase C: xp1 = y0 @ W_ih[1]^T + bias1   (K=1024)
        # ------------------------------------------------------------------
        with (
            tc.tile_pool(name="c_w", bufs=1) as wpool,
            tc.tile_pool(name="c_const", bufs=1) as cpool,
            tc.tile_pool(name="c_stat", bufs=3) as spool,
            tc.tile_pool(name="c_ps", bufs=2, space="PSUM") as pspool,
            tc.tile_pool(name="c_out", bufs=3) as opool,
        ):
            w1_sb = wpool.tile([128, NK * G], F32R)
            for c in range(NK):
                nc.sync.dma_start(
                    out=w1_sb[:, c * G : (c + 1) * G],
                    in_=w1T[c * 128 : (c + 1) * 128, :],
                )
            bias_sb = cpool.tile([128, G], F32)
            nc.sync.dma_start(out=bias_sb[:], in_=bias128[1])
            for rt in range(nrt):
                st = spool.tile([128, H], F32R)
                for c in range(NK):
                    nc.sync.dma_start(
                        out=st[:, c * 128 : (c + 1) * 128],
                        in_=y0T_d[c, :, rt * 128 : (rt + 1) * 128],
                    )
                for half in range(2):
                    ps = pspool.tile([128, G // 2], F32)
                    for j in range(NNC // 2):
                        col = half * (G // 2) + j * 512
                        for k in range(NK):
                            nc.tensor.matmul(
                                ps[:, j * 512 : (j + 1) * 512],
                                st[:, k * 128 : (k + 1) * 128],
                                w1_sb[:, k * G + col : k * G + col + 512],
                                start=(k == 0),
                                stop=(k == NK - 1),
                            )
                    ot = opool.tile([128, G // 2], F32, tag="c_out")
                    nc.vector.tensor_add(
                        ot[:], ps[:], bias_sb[:, half * (G // 2) : (half + 1) * (G // 2)]
                    )
                    nc.sync.dma_start(
                        out=xp1_d[
                            rt * 128 : (rt + 1) * 128,
                            half * (G // 2) : (half + 1) * (G // 2),
                        ],
                        in_=ot[:],
                    )

        recurrence(1, xp1_d, y1T_d)

        # ------------------------------------------------------------------
        # phase E: logits = y1 @ sm_fc_w^T   (N=64, sm_fc_b is zero)
        # ------------------------------------------------------------------
        with (
            tc.tile_pool(name="e_const", bufs=1) as cpool,
            tc.tile_pool(name="e_stat", bufs=3) as spool,
            tc.tile_pool(name="e_ps", bufs=2, space="PSUM") as pspool,
            tc.tile_pool(name="e_out", bufs=3) as opool,
        ):
            smw_sb = cpool.tile([128, NK * I], F32R)
            for c in range(NK):
                nc.sync.dma_start(
                    out=smw_sb[:, c * I : (c + 1) * I],
                    in_=smwT[c * 128 : (c + 1) * 128, :],
                )
            for rt in range(nrt):
                st = spool.tile([128, H], F32R, tag="e_stat")
                for c in range(NK):
                    nc.sync.dma_start(
                        out=st[:, c * 128 : (c + 1) * 128],
                        in_=y1T_d[c, :, rt * 128 : (rt + 1) * 128],
                    )
                ps = pspool.tile([128, I], F32)
                for k in range(NK):
                    nc.tensor.matmul(
                        ps[:],
                        st[:, k * 128 : (k + 1) * 128],
                        smw_sb[:, k * I : (k + 1) * I],
                        start=(k == 0),
                        stop=(k == NK - 1),
                    )
                ot = opool.tile([128, I], F32, tag="e_out")
                nc.vector.tensor_copy(ot[:], ps[:])
                nc.sync.dma_start(
                    out=logits_o[rt * 128 : (rt + 1) * 128, :], in_=ot[:]
                )

    return nc


_program_cache = {}


def _get_program(nsteps):
    if nsteps not in _program_cache:
        _program_cache[nsteps] = build_program(nsteps)
    return _program_cache[nsteps]


def _host_prep(inputs, hidden, in_fc_w, in_fc_b, w_ih, w_hh, b_ih, b_hh,
               sm_fc_w, sm_fc_b, nsteps):
    f = np.float32
    w0eff = (w_ih[0] @ in_fc_w).astype(f)                      # [3H, I]
    # in_fc_b is folded through W_ih[0] too (reference: x0 = x@Wfc^T + b_fc)
    extra0 = (w_ih[0] @ in_fc_b).astype(f)                     # [3H]
    bias = []
    for l in range(L):
        br = b_ih[l] + np.concatenate([b_hh[l][: 2 * H], np.zeros(H, f)])
        if l == 0:
            br = br + extra0
        bias.append(np.tile(br.astype(f)[None, :], (128, 1)))
    bias128 = np.stack(bias)                                   # [L,128,3H]
    bhh_rep = np.stack(
        [np.tile(b_hh[l][2 * H :].astype(f)[None, :], (BLOC, 1)) for l in range(L)]
    )                                                          # [L,BLOC,H]
    shared = {
        "w0effT": np.ascontiguousarray(w0eff.T),
        "w1T": np.ascontiguousarray(w_ih[1].T.astype(f)),
        "whhT": np.ascontiguousarray(w_hh.transpose(0, 2, 1).astype(f)),
        "bias128": np.ascontiguousarray(bias128),
        "bhh_rep": np.ascontiguousarray(bhh_rep),
        "smwT": np.ascontiguousarray(sm_fc_w.T.astype(f)),
    }
    in_maps = []
    for c in range(NCORES):
        bs = slice(c * BLOC, (c + 1) * BLOC)
        xc = inputs[:nsteps, bs, :].astype(f)                  # [S,BLOC,I]
        xT = np.ascontiguousarray(xc.transpose(2, 0, 1).reshape(I, nsteps * BLOC))
        h0c = np.ascontiguousarray(hidden[:, bs, :].astype(f))  # [L,BLOC,H]
        h0T = np.ascontiguousarray(h0c.transpose(0, 2, 1))      # [L,H,BLOC]
        m = dict(shared)
        m["xT"] = xT
        m["h0"] = h0c
        m["h0T"] = h0T
        in_maps.append(m)
    return in_maps


def kernel(inputs, hidden, in_fc_w, in_fc_b, w_ih, w_hh, b_ih, b_hh,
           sm_fc_w, sm_fc_b, nsteps=S, trace=False, tmpdir=None):
    inputs = np.asarray(inputs, np.float32)
    hidden = np.asarray(hidden, np.float32)
    args = [np.asarray(a, np.float32) for a in
            (in_fc_w, in_fc_b, w_ih, w_hh, b_ih, b_hh, sm_fc_w, sm_fc_b)]
    nc = _get_program(nsteps)
    in_maps = _host_prep(inputs, hidden, *args, nsteps)
    res = run_bass_kernel_spmd(
        nc, in_maps, list(range(NCORES)), trace=trace, tmpdir=tmpdir
    )
    kernel.last_result = res
    logits = np.concatenate(
        [res.results[c]["logits"].reshape(nsteps, BLOC, I) for c in range(NCORES)],
        axis=1,
    )
    hidden_out = np.concatenate(
        [res.results[c]["hidden"] for c in range(NCORES)], axis=1
    )
    # sm_fc_b is zero in the reference setup but add for generality
    logits = logits + np.asarray(sm_fc_b, np.float32)[None, None, :]
    return logits, hidden_out
